# revision 15
# baseline (speedup 1.0000x reference)
"""CAPMemory loss kernel for 8 Trainium2 NeuronCores.

Sharding: camera-sharded -- core c owns memory[c], the batch is replicated
(the per-sample stats each core produces are tiny, so this moves 16x less
HBM traffic than batch-sharding the replicated 128 MiB memory bank).

Per-core batch permutation: core c receives the batch reordered so its
own-camera samples occupy rows 0..cnt_c-1 (cnt_c <= 256 w.h.p.).  The
intra-loss denominator zin = sum_l exp(S/T) is then only needed for batch
tiles 0-1, so the device computes exp for those two tiles only.

Device, per core (fp8 e4m3 DoubleRow matmul, fp32 PSUM):
  S[b, l] = <x_norm[b], memory[c, l]> * FP8_SCALE^2      [1024, 2048]
  per 512-col PSUM bank: E = exp(S / (FP8_SCALE^2 * T))  (ACT, bf16)
    - btiles 0,1: with free-dim accumulate -> zin partials
  cand = top-8 of each 256-wide chunk of E -> 64 values/sample (DVE MAX8)

Matmul schedule: btiles 0,1 accumulate into the two PSUM buffers with
matmuls interleaved in input-chunk-arrival order (the input DMA is the
gate early on); btiles 2-7 run bank-major (all 8 contraction chunks of
one 512-col bank back to back) so each PSUM bank is released to the
next btile as early as possible via its per-bank exp.

Host merge:
  epos[c, b] = exp(<x8[b], m8[c, tgt_b]>/T') recomputed in f32 from the
  exact fp8 operands the device consumed; intra CE = log(zin) - log(epos)
  on the own-camera core (host fallback matvec for the vanishingly rare
  cnt_c > 256 overflow rows).  For the inter loss the positive's value is
  removed from its camera's candidate list (nearest match to epos), the
  8x64 candidates are merged, and the exact top-50 negatives feed the
  log-sum-exp.  A global top-50 element can only be missing from the
  candidates if >=8 larger elements share its 256-chunk (P ~ 1e-5 per
  run, and the substitute is the next-ranked value, so the effect is
  ~1e-6 relative even then).
"""

import numpy as np

T = 0.05
HARD_NEG_K = 50
LOSS_WEIGHT = 0.5
N_CAMS = 8
L = 2048
D = 2048
B = 1024
NBT = 8          # batch tiles of 128
KC8 = 8          # contraction chunks of 256 (fp8 DoubleRow: 2 k-rows/cell)
FP8_SCALE = 32.0  # pre-scale before e4m3 cast (keeps values out of denormals)
NCH = 8          # candidate chunks per row
CHW = 256        # chunk width
NTOP = NCH * 8   # candidates shipped per camera (top-8 of each chunk)
NZT = 2          # btiles with exp+zin (own-camera rows live in tiles 0-1)
NWARM = 11       # PE warm-up matmuls (~4us; bridges until the first input chunk)

_CACHE = {}


def _split_multi_waits(nc):
    """This container's walrus build rejects instructions carrying more than
    one sync wait ('Too many sync wait commands'). Hoist all but the last
    wait of each instruction onto same-engine Drain carriers placed just
    before it — semantically identical on an in-order engine stream."""
    import concourse.mybir as mybir

    n = 0
    for fn in nc.m.functions:
        for bb in fn.blocks:
            out = []
            for inst in bb.instructions:
                si = inst.sync_info
                if si is not None and si.on_wait and len(si.on_wait) > 1:
                    waits = list(si.on_wait)
                    for w in waits[:-1]:
                        d = mybir.InstDrain(name=f"ws-{n}", ins=[], outs=[])
                        n += 1
                        d.engine = inst.engine
                        d.sync_info = mybir.SyncInfo(on_wait=[w], on_update=[])
                        out.append(d)
                    si.on_wait = [waits[-1]]
                out.append(inst)
            if n:
                bb.instructions = out


def _build():
    import concourse.bass as bass
    import concourse.mybir as mybir
    from concourse import tile

    f32 = mybir.dt.float32
    bf16 = mybir.dt.bfloat16
    f8 = mybir.dt.float8e4
    Act = mybir.ActivationFunctionType

    nc = bass.Bass()
    xT = nc.dram_tensor("xT", [KC8, 128, 2, B], f8, kind="ExternalInput")
    mT = nc.dram_tensor("mT", [KC8, 128, 2, L], f8, kind="ExternalInput")
    zin_d = nc.dram_tensor("zin", [128, NZT], f32, kind="ExternalOutput")
    topv_d = nc.dram_tensor("topv", [NBT, 128, NTOP], bf16, kind="ExternalOutput")

    ESCALE = 1.0 / (FP8_SCALE * FP8_SCALE * T)

    with tile.TileContext(nc) as tc:
        with (
            tc.tile_pool(name="const", bufs=1) as cpool,
            tc.tile_pool(name="psum", bufs=8, space="PSUM") as ppool,
            tc.tile_pool(name="work", bufs=3) as wpool,
            tc.tile_pool(name="small", bufs=3) as spool,
        ):
            X = cpool.tile([128, KC8, 2, B], f8)
            M = cpool.tile([128, KC8, 2, L], f8)
            # DMA issue order sets the HW queue-lane pairing (8 lanes, two
            # transfers each).  Chunks 0-3 go as (x,m) pairs so the fill
            # matmuls start early; the second wave is paced via explicit
            # deps on fill-matmul progress (below) so arrivals stay roughly
            # in consumption order instead of bunching at the tail.
            in_trigs = {}
            for kc in range(4):
                in_trigs[("x", kc)] = nc.sync.dma_start(X[:, kc, :, :], xT[kc])
                in_trigs[("m", kc)] = nc.sync.dma_start(M[:, kc, :, :], mT[kc])
            for kc in range(4, KC8):
                in_trigs[("m", kc)] = nc.sync.dma_start(M[:, kc, :, :], mT[kc])
            for kc in range(4, KC8):
                in_trigs[("x", kc)] = nc.sync.dma_start(X[:, kc, :, :], xT[kc])
            ZIN = cpool.tile([128, NZT], f32)

            # PE warm-up: HAM needs ~3-4us of sustained activity to reach
            # 2.4 GHz. Run throwaway matmuls on a zeroed scratch tile while
            # the input DMAs are still in flight; sized so the warm-up ends
            # about when the first input chunk pair lands (~13us).
            GB = cpool.tile([128, 640], f8)
            nc.vector.memset(GB[:], 0.0)
            WARM = ppool.tile([128, 512], f32, tag="S")
            for _ in range(NWARM):
                nc.tensor.matmul(
                    WARM[:], GB[:, 0:128], GB[:, 128:640],
                    start=True, stop=True,
                )

            # PSUM is managed as 8 single-bank [128,512] tiles: a btile's S
            # occupies 4 of them, and the pool rotation makes btile j+2's
            # bank-k matmuls wait only on the exp of btile j's bank k (not
            # on the whole previous tile's reader set).
            def s_banks():
                return [
                    ppool.tile([128, 512], f32, tag="S", name=f"sb{k}")
                    for k in range(4)
                ]

            # btiles 0 and 1 accumulate into both PSUM tile groups with
            # their matmuls interleaved in chunk-arrival order: during the
            # input DMA fill each arriving (X,M) chunk feeds 8 ready matmuls
            S_a = s_banks()
            S_b = s_banks()
            S_pair = [S_a, S_b]
            fill_mms = []
            for i in range(KC8):
                for bt in range(2):
                    for nch in range(4):
                        fill_mms.append(nc.tensor.matmul(
                            S_pair[bt][nch][:],
                            X[:, i, :, bt * 128 : (bt + 1) * 128],
                            M[:, i, :, nch * 512 : (nch + 1) * 512],
                            start=(i == 0),
                            stop=(i == KC8 - 1),
                            perf_mode=mybir.MatmulPerfMode.DoubleRow,
                        ))

            # pace the second input-DMA wave off fill progress: chunk k's
            # transfer is triggered ~3 chunks ahead of its consumption, so
            # the HBM stays saturated but arrivals land just-in-time and
            # the PE never idles long enough to re-throttle.
            pace = {
                ("m", 4): 12, ("x", 4): 16,
                ("m", 5): 20, ("x", 5): 24,
                ("m", 6): 28, ("x", 6): 30,
                ("m", 7): 32, ("x", 7): 32,
            }
            for key, mi in pace.items():
                tile.add_dep_helper(
                    in_trigs[key].ins, fill_mms[mi].ins,
                    reason="pace wave-2 input DMA off fill matmul progress",
                )

            def exp_and_mine(S, bt):
                # per-512-bank exp (ACT, no accumulator -> no READ_ACC
                # serialization chain) so PSUM banks free progressively;
                # zin for the own-camera btiles 0-1 comes from a DVE
                # free-dim reduce over E instead.
                E = wpool.tile([128, L], bf16, tag="E")
                for nch in range(4):
                    sl = slice(nch * 512, (nch + 1) * 512)
                    nc.scalar.activation(
                        E[:, sl], S[nch][:], Act.Exp, scale=ESCALE,
                    )
                cand = spool.tile([128, NCH * 8], bf16, tag="cand")
                for ch in range(NCH):
                    nc.vector.max(
                        cand[:, ch * 8 : (ch + 1) * 8],
                        E[:, ch * CHW : (ch + 1) * CHW],
                    )
                nc.sync.dma_start(topv_d[bt], cand[:])
                if bt < NZT:
                    nc.vector.reduce_sum(
                        ZIN[:, bt : bt + 1], E[:], axis=mybir.AxisListType.X
                    )
                    if bt == NZT - 1:
                        nc.sync.dma_start(zin_d[:], ZIN[:])

            exp_and_mine(S_a, 0)
            exp_and_mine(S_b, 1)

            # btiles 2-7: bank-major matmul order (all 8 contraction chunks
            # of one 512-col bank back to back) so each bank's exp can fire
            # ~1.7us after the btile starts, releasing PSUM to btile+2
            # exactly when its matmuls reach that bank.
            for bt in range(2, NBT):
                S = s_banks()
                for nch in range(4):
                    for i, kc in enumerate([(k + bt) % KC8 for k in range(KC8)]):
                        nc.tensor.matmul(
                            S[nch][:],
                            X[:, kc, :, bt * 128 : (bt + 1) * 128],
                            M[:, kc, :, nch * 512 : (nch + 1) * 512],
                            start=(i == 0),
                            stop=(i == KC8 - 1),
                            perf_mode=mybir.MatmulPerfMode.DoubleRow,
                        )
                exp_and_mine(S, bt)

    _split_multi_waits(nc)
    return nc


def _get_nc():
    if "nc" not in _CACHE:
        _CACHE["nc"] = _build()
    return _CACHE["nc"]


def _pack_fp8(aT, ncols, f8):
    # [D, n] -> [KC8, 128, 2, n] with d = kc*256 + j*128 + p
    v = np.clip(aT * FP8_SCALE, -240.0, 240.0)
    v = v.reshape(KC8, 2, 128, ncols).transpose(0, 2, 1, 3)
    return np.ascontiguousarray(v).astype(f8)


def _prepare(inputs, memory, indexes, cams_all, labels_all):
    import ml_dtypes

    f8 = ml_dtypes.float8_e4m3
    inputs = np.asarray(inputs, np.float32)
    memory = np.asarray(memory, np.float32)
    indexes = np.asarray(indexes).astype(np.int64)
    cams_all = np.asarray(cams_all).astype(np.int64)
    cams = cams_all[indexes]

    x = inputs / np.linalg.norm(inputs, axis=1, keepdims=True)
    # per-core batch permutation: own-camera samples first (stable order)
    perms = [np.argsort(cams != c, kind="stable") for c in range(N_CAMS)]
    in_maps = []
    for c in range(N_CAMS):
        xTc = _pack_fp8(np.ascontiguousarray(x[perms[c]].T), B, f8)
        mTc = _pack_fp8(memory[c].T, L, f8)
        in_maps.append({"xT": xTc, "mT": mTc})
    return in_maps, perms, cams


def kernel(inputs, memory, indexes, cams_all, labels_all):
    from concourse.bass_utils import run_bass_kernel_spmd

    indexes = np.asarray(indexes).astype(np.int64)
    cams_all = np.asarray(cams_all).astype(np.int64)
    labels_all = np.asarray(labels_all).astype(np.int64)

    in_maps, perms, cams = _prepare(inputs, memory, indexes, cams_all, labels_all)
    nc = _get_nc()
    res = run_bass_kernel_spmd(nc, in_maps, list(range(N_CAMS)))

    # epos = exp(S[t]/T) computed host-side from the same fp8-quantized
    # inputs the device consumed (f32 arithmetic ~= PSUM fp32 accumulate).
    # x8/m8 reconstructed in the ORIGINAL batch order.
    tgts = labels_all[indexes]
    x_norm = np.asarray(inputs, np.float32)
    x_norm = x_norm / np.linalg.norm(x_norm, axis=1, keepdims=True)
    x8 = np.clip(x_norm.T * FP8_SCALE, -240.0, 240.0)
    x8 = x8.astype(in_maps[0]["xT"].dtype).astype(np.float32)   # [D, B]
    epos = np.empty((N_CAMS, B), np.float64)
    m8s = []
    for c in range(N_CAMS):
        m8 = in_maps[c]["mT"].transpose(0, 2, 1, 3).reshape(D, L).astype(np.float32)
        m8s.append(m8)
        mt = m8[:, tgts]                     # [D, B]
        s_t = np.einsum("db,db->b", x8, mt, optimize=True)
        epos[c] = np.exp(s_t.astype(np.float64) / (FP8_SCALE * FP8_SCALE * T))

    bidx = np.arange(B)

    # gather per-core stats; zin rows are the first 2*128 rows of core c's
    # permuted batch; topv rows map back through the permutation
    zin_dev = np.empty((N_CAMS, NZT * 128), np.float64)
    topv = np.empty((N_CAMS, B, NTOP), np.float64)
    for c in range(N_CAMS):
        r = res.results[c]
        zin_dev[c] = r["zin"].astype(np.float64).T.reshape(NZT * 128)
        tv = r["topv"].astype(np.float64).reshape(B, NTOP)   # permuted rows
        inv = np.empty(B, np.int64)
        inv[perms[c]] = bidx
        topv[c] = tv[inv]                                    # original order

    # ---- intra: CE against own camera, mean within camera group, summed
    zin_own = np.empty(B, np.float64)
    for c in range(N_CAMS):
        own = np.flatnonzero(cams == c)                      # == perms[c][:cnt]
        rows = np.empty(B, np.int64)
        rows[perms[c]] = bidx                                # row of b in perm order
        r_own = rows[own]
        ok = r_own < NZT * 128
        zin_own[own[ok]] = zin_dev[c][r_own[ok]]
        for b in own[~ok]:                                   # overflow fallback
            s_row = x8[:, b] @ m8s[c]
            zin_own[b] = np.exp(
                s_row.astype(np.float64) / (FP8_SCALE * FP8_SCALE * T)
            ).sum()
    epos_own = epos[cams, bidx]
    ce = np.log(zin_own) - np.log(epos_own)
    cnt = np.bincount(cams, minlength=N_CAMS).astype(np.float64)
    ce_sum = np.bincount(cams, weights=ce, minlength=N_CAMS)
    loss_intra = np.sum(ce_sum / np.maximum(cnt, 1.0))

    # remove the positive's own value from each camera's candidate list:
    # nearest candidate within 0.5% of the host-computed epos (device values
    # are bf16-rounded, so exact equality is not available)
    for c in range(N_CAMS):
        relerr = np.abs(topv[c] - epos[c][:, None]) / epos[c][:, None]
        j = np.argmin(relerr, axis=1)
        hit = relerr[bidx, j] < 5e-3
        topv[c][bidx[hit], j[hit]] = 0.0

    # ---- inter: exact global top-50 negatives from 8x64 candidates
    cand = topv.transpose(1, 0, 2).reshape(B, N_CAMS * NTOP)
    part = np.partition(cand, cand.shape[1] - HARD_NEG_K, axis=1)
    z50 = part[:, cand.shape[1] - HARD_NEG_K :].sum(axis=1)
    sum_epos = epos.sum(axis=0)
    lse = np.log(sum_epos + z50)
    mean_logpos = np.log(epos).mean(axis=0)
    per_sample = lse - mean_logpos
    inter_sum = np.bincount(cams, weights=per_sample, minlength=N_CAMS)
    loss_inter = np.sum(inter_sum / np.maximum(cnt, 1.0)) * LOSS_WEIGHT

    return np.float32(loss_intra), np.float32(loss_inter)


# revision 18
# speedup vs baseline: 1.0758x; 1.0758x over previous
"""CAPMemory loss kernel for 8 Trainium2 NeuronCores.

Sharding: camera-sharded -- core c owns memory[c], the batch is replicated
(the per-sample stats each core produces are tiny, so this moves 16x less
HBM traffic than batch-sharding the replicated 128 MiB memory bank).

Per-core batch permutation: core c receives the batch reordered so its
own-camera samples occupy rows 0..cnt_c-1 (cnt_c <= 256 w.h.p.).  The
intra-loss denominator zin = sum_l exp(S/T) is then only needed for batch
tiles 0-1, so the device computes exp for those two tiles only.

Device, per core (fp8 e4m3 DoubleRow matmul, fp32 PSUM):
  S[b, l] = <x_norm[b], memory[c, l]> * FP8_SCALE^2      [1024, 2048]
  per 512-col PSUM bank: E = exp(S / (FP8_SCALE^2 * T))  (ACT, bf16)
    - btiles 0,1: with free-dim accumulate -> zin partials
  cand = top-8 of each 256-wide chunk of E -> 64 values/sample (DVE MAX8)

Matmul schedule: btiles 0,1 accumulate into the two PSUM buffers with
matmuls interleaved in input-chunk-arrival order (the input DMA is the
gate early on); btiles 2-7 run bank-major (all 8 contraction chunks of
one 512-col bank back to back) so each PSUM bank is released to the
next btile as early as possible via its per-bank exp.

Host merge:
  epos[c, b] = exp(<x8[b], m8[c, tgt_b]>/T') recomputed in f32 from the
  exact fp8 operands the device consumed; intra CE = log(zin) - log(epos)
  on the own-camera core (host fallback matvec for the vanishingly rare
  cnt_c > 256 overflow rows).  For the inter loss the positive's value is
  removed from its camera's candidate list (nearest match to epos), the
  8x64 candidates are merged, and the exact top-50 negatives feed the
  log-sum-exp.  A global top-50 element can only be missing from the
  candidates if >=8 larger elements share its 256-chunk (P ~ 1e-5 per
  run, and the substitute is the next-ranked value, so the effect is
  ~1e-6 relative even then).
"""

import numpy as np

T = 0.05
HARD_NEG_K = 50
LOSS_WEIGHT = 0.5
N_CAMS = 8
L = 2048
D = 2048
B = 1024
NBT = 8          # batch tiles of 128
KC8 = 8          # contraction chunks of 256 (fp8 DoubleRow: 2 k-rows/cell)
FP8_SCALE = 32.0  # pre-scale before e4m3 cast (keeps values out of denormals)
NCH = 8          # candidate chunks per row
CHW = 256        # chunk width
NTOP = NCH * 8   # candidates shipped per camera (top-8 of each chunk)
NZT = 2          # btiles with exp+zin (own-camera rows live in tiles 0-1)
NWARM = 6        # PE warm-up matmuls (~2.6us; first input chunk lands ~11us)

_CACHE = {}


def _split_multi_waits(nc):
    """This container's walrus build rejects instructions carrying more than
    one sync wait ('Too many sync wait commands'). Hoist all but the last
    wait of each instruction onto same-engine Drain carriers placed just
    before it — semantically identical on an in-order engine stream."""
    import concourse.mybir as mybir

    n = 0
    for fn in nc.m.functions:
        for bb in fn.blocks:
            out = []
            for inst in bb.instructions:
                si = inst.sync_info
                if si is not None and si.on_wait and len(si.on_wait) > 1:
                    waits = list(si.on_wait)
                    for w in waits[:-1]:
                        d = mybir.InstDrain(name=f"ws-{n}", ins=[], outs=[])
                        n += 1
                        d.engine = inst.engine
                        d.sync_info = mybir.SyncInfo(on_wait=[w], on_update=[])
                        out.append(d)
                    si.on_wait = [waits[-1]]
                out.append(inst)
            if n:
                bb.instructions = out


def _build():
    import concourse.bass as bass
    import concourse.mybir as mybir
    from concourse import tile

    f32 = mybir.dt.float32
    bf16 = mybir.dt.bfloat16
    f8 = mybir.dt.float8e4
    Act = mybir.ActivationFunctionType

    nc = bass.Bass()
    xT = nc.dram_tensor("xT", [KC8, 128, 2, B], f8, kind="ExternalInput")
    mT = nc.dram_tensor("mT", [KC8, 128, 2, L], f8, kind="ExternalInput")
    zin_d = nc.dram_tensor("zin", [128, NZT], f32, kind="ExternalOutput")
    topv_d = nc.dram_tensor("topv", [NBT, 128, NTOP], bf16, kind="ExternalOutput")

    ESCALE = 1.0 / (FP8_SCALE * FP8_SCALE * T)

    with tile.TileContext(nc) as tc:
        with (
            tc.tile_pool(name="const", bufs=1) as cpool,
            tc.tile_pool(name="psum", bufs=8, space="PSUM") as ppool,
            tc.tile_pool(name="work", bufs=3) as wpool,
            tc.tile_pool(name="small", bufs=3) as spool,
        ):
            X = cpool.tile([128, KC8, 2, B], f8)
            M = cpool.tile([128, KC8, 2, L], f8)
            # All input transfers land on one HW queue and are serviced
            # serially in trigger order, so (x,m) pair order IS the arrival
            # order: chunk k lands at ~11 + 2.15k us, just ahead of the
            # fill matmuls that consume it.
            for kc in range(KC8):
                nc.sync.dma_start(X[:, kc, :, :], xT[kc])
                nc.sync.dma_start(M[:, kc, :, :], mT[kc])
            ZIN = cpool.tile([128, NZT], f32)

            # PE warm-up: HAM needs ~3-4us of sustained activity to reach
            # 2.4 GHz. Run throwaway matmuls on a zeroed scratch tile while
            # the input DMAs are still in flight; sized so the warm-up ends
            # about when the first input chunk pair lands (~13us).
            GB = cpool.tile([128, 640], f8)
            nc.vector.memset(GB[:], 0.0)
            WARM = ppool.tile([128, 512], f32, tag="S")
            for _ in range(NWARM):
                nc.tensor.matmul(
                    WARM[:], GB[:, 0:128], GB[:, 128:640],
                    start=True, stop=True,
                )

            # PSUM is managed as 8 single-bank [128,512] tiles: a btile's S
            # occupies 4 of them, and the pool rotation makes btile j+2's
            # bank-k matmuls wait only on the exp of btile j's bank k (not
            # on the whole previous tile's reader set).
            def s_banks():
                return [
                    ppool.tile([128, 512], f32, tag="S", name=f"sb{k}")
                    for k in range(4)
                ]

            # btiles 0 and 1 accumulate into both PSUM tile groups with
            # their matmuls interleaved in chunk-arrival order: during the
            # input DMA fill each arriving (X,M) chunk feeds 8 ready matmuls
            S_a = s_banks()
            S_b = s_banks()
            S_pair = [S_a, S_b]
            for i in range(KC8):
                for bt in range(2):
                    for nch in range(4):
                        nc.tensor.matmul(
                            S_pair[bt][nch][:],
                            X[:, i, :, bt * 128 : (bt + 1) * 128],
                            M[:, i, :, nch * 512 : (nch + 1) * 512],
                            start=(i == 0),
                            stop=(i == KC8 - 1),
                            perf_mode=mybir.MatmulPerfMode.DoubleRow,
                        )

            def exp_and_mine(S, bt):
                # per-512-bank exp (ACT, no accumulator -> no READ_ACC
                # serialization chain) so PSUM banks free progressively;
                # zin for the own-camera btiles 0-1 comes from a DVE
                # free-dim reduce over E instead.
                E = wpool.tile([128, L], bf16, tag="E")
                for nch in range(4):
                    sl = slice(nch * 512, (nch + 1) * 512)
                    nc.scalar.activation(
                        E[:, sl], S[nch][:], Act.Exp, scale=ESCALE,
                    )
                cand = spool.tile([128, NCH * 8], bf16, tag="cand")
                for ch in range(NCH):
                    nc.vector.max(
                        cand[:, ch * 8 : (ch + 1) * 8],
                        E[:, ch * CHW : (ch + 1) * CHW],
                    )
                nc.sync.dma_start(topv_d[bt], cand[:])
                if bt < NZT:
                    nc.vector.reduce_sum(
                        ZIN[:, bt : bt + 1], E[:], axis=mybir.AxisListType.X
                    )
                    if bt == NZT - 1:
                        nc.sync.dma_start(zin_d[:], ZIN[:])

            exp_and_mine(S_a, 0)
            exp_and_mine(S_b, 1)

            # btiles 2-7: bank-major matmul order (all 8 contraction chunks
            # of one 512-col bank back to back) so each bank's exp can fire
            # ~1.7us after the btile starts, releasing PSUM to btile+2
            # exactly when its matmuls reach that bank.
            for bt in range(2, NBT):
                S = s_banks()
                for nch in range(4):
                    for i, kc in enumerate([(k + bt) % KC8 for k in range(KC8)]):
                        nc.tensor.matmul(
                            S[nch][:],
                            X[:, kc, :, bt * 128 : (bt + 1) * 128],
                            M[:, kc, :, nch * 512 : (nch + 1) * 512],
                            start=(i == 0),
                            stop=(i == KC8 - 1),
                            perf_mode=mybir.MatmulPerfMode.DoubleRow,
                        )
                exp_and_mine(S, bt)

    _split_multi_waits(nc)
    return nc


def _get_nc():
    if "nc" not in _CACHE:
        _CACHE["nc"] = _build()
    return _CACHE["nc"]


def _pack_fp8(aT, ncols, f8):
    # [D, n] -> [KC8, 128, 2, n] with d = kc*256 + j*128 + p
    v = np.clip(aT * FP8_SCALE, -240.0, 240.0)
    v = v.reshape(KC8, 2, 128, ncols).transpose(0, 2, 1, 3)
    return np.ascontiguousarray(v).astype(f8)


def _prepare(inputs, memory, indexes, cams_all, labels_all):
    import ml_dtypes

    f8 = ml_dtypes.float8_e4m3
    inputs = np.asarray(inputs, np.float32)
    memory = np.asarray(memory, np.float32)
    indexes = np.asarray(indexes).astype(np.int64)
    cams_all = np.asarray(cams_all).astype(np.int64)
    cams = cams_all[indexes]

    x = inputs / np.linalg.norm(inputs, axis=1, keepdims=True)
    # per-core batch permutation: own-camera samples first (stable order)
    perms = [np.argsort(cams != c, kind="stable") for c in range(N_CAMS)]
    in_maps = []
    for c in range(N_CAMS):
        xTc = _pack_fp8(np.ascontiguousarray(x[perms[c]].T), B, f8)
        mTc = _pack_fp8(memory[c].T, L, f8)
        in_maps.append({"xT": xTc, "mT": mTc})
    return in_maps, perms, cams


def kernel(inputs, memory, indexes, cams_all, labels_all):
    from concourse.bass_utils import run_bass_kernel_spmd

    indexes = np.asarray(indexes).astype(np.int64)
    cams_all = np.asarray(cams_all).astype(np.int64)
    labels_all = np.asarray(labels_all).astype(np.int64)

    in_maps, perms, cams = _prepare(inputs, memory, indexes, cams_all, labels_all)
    nc = _get_nc()
    res = run_bass_kernel_spmd(nc, in_maps, list(range(N_CAMS)))

    # epos = exp(S[t]/T) computed host-side from the same fp8-quantized
    # inputs the device consumed (f32 arithmetic ~= PSUM fp32 accumulate).
    # x8/m8 reconstructed in the ORIGINAL batch order.
    tgts = labels_all[indexes]
    x_norm = np.asarray(inputs, np.float32)
    x_norm = x_norm / np.linalg.norm(x_norm, axis=1, keepdims=True)
    x8 = np.clip(x_norm.T * FP8_SCALE, -240.0, 240.0)
    x8 = x8.astype(in_maps[0]["xT"].dtype).astype(np.float32)   # [D, B]
    epos = np.empty((N_CAMS, B), np.float64)
    m8s = []
    for c in range(N_CAMS):
        m8 = in_maps[c]["mT"].transpose(0, 2, 1, 3).reshape(D, L).astype(np.float32)
        m8s.append(m8)
        mt = m8[:, tgts]                     # [D, B]
        s_t = np.einsum("db,db->b", x8, mt, optimize=True)
        epos[c] = np.exp(s_t.astype(np.float64) / (FP8_SCALE * FP8_SCALE * T))

    bidx = np.arange(B)

    # gather per-core stats; zin rows are the first 2*128 rows of core c's
    # permuted batch; topv rows map back through the permutation
    zin_dev = np.empty((N_CAMS, NZT * 128), np.float64)
    topv = np.empty((N_CAMS, B, NTOP), np.float64)
    for c in range(N_CAMS):
        r = res.results[c]
        zin_dev[c] = r["zin"].astype(np.float64).T.reshape(NZT * 128)
        tv = r["topv"].astype(np.float64).reshape(B, NTOP)   # permuted rows
        inv = np.empty(B, np.int64)
        inv[perms[c]] = bidx
        topv[c] = tv[inv]                                    # original order

    # ---- intra: CE against own camera, mean within camera group, summed
    zin_own = np.empty(B, np.float64)
    for c in range(N_CAMS):
        own = np.flatnonzero(cams == c)                      # == perms[c][:cnt]
        rows = np.empty(B, np.int64)
        rows[perms[c]] = bidx                                # row of b in perm order
        r_own = rows[own]
        ok = r_own < NZT * 128
        zin_own[own[ok]] = zin_dev[c][r_own[ok]]
        for b in own[~ok]:                                   # overflow fallback
            s_row = x8[:, b] @ m8s[c]
            zin_own[b] = np.exp(
                s_row.astype(np.float64) / (FP8_SCALE * FP8_SCALE * T)
            ).sum()
    epos_own = epos[cams, bidx]
    ce = np.log(zin_own) - np.log(epos_own)
    cnt = np.bincount(cams, minlength=N_CAMS).astype(np.float64)
    ce_sum = np.bincount(cams, weights=ce, minlength=N_CAMS)
    loss_intra = np.sum(ce_sum / np.maximum(cnt, 1.0))

    # remove the positive's own value from each camera's candidate list:
    # nearest candidate within 0.5% of the host-computed epos (device values
    # are bf16-rounded, so exact equality is not available)
    for c in range(N_CAMS):
        relerr = np.abs(topv[c] - epos[c][:, None]) / epos[c][:, None]
        j = np.argmin(relerr, axis=1)
        hit = relerr[bidx, j] < 5e-3
        topv[c][bidx[hit], j[hit]] = 0.0

    # ---- inter: exact global top-50 negatives from 8x64 candidates
    cand = topv.transpose(1, 0, 2).reshape(B, N_CAMS * NTOP)
    part = np.partition(cand, cand.shape[1] - HARD_NEG_K, axis=1)
    z50 = part[:, cand.shape[1] - HARD_NEG_K :].sum(axis=1)
    sum_epos = epos.sum(axis=0)
    lse = np.log(sum_epos + z50)
    mean_logpos = np.log(epos).mean(axis=0)
    per_sample = lse - mean_logpos
    inter_sum = np.bincount(cams, weights=per_sample, minlength=N_CAMS)
    loss_inter = np.sum(inter_sum / np.maximum(cnt, 1.0)) * LOSS_WEIGHT

    return np.float32(loss_intra), np.float32(loss_inter)


# revision 23
# speedup vs baseline: 1.0795x; 1.0034x over previous
"""CAPMemory loss kernel for 8 Trainium2 NeuronCores.

Sharding: camera-sharded -- core c owns memory[c], the batch is replicated
(the per-sample stats each core produces are tiny, so this moves 16x less
HBM traffic than batch-sharding the replicated 128 MiB memory bank).

Per-core batch permutation: core c receives the batch reordered so its
own-camera samples occupy rows 0..cnt_c-1 (cnt_c <= 256 w.h.p.).  The
intra-loss denominator zin = sum_l exp(S/T) is then only needed for batch
tiles 0-1, so the device computes exp for those two tiles only.

Device, per core (fp8 e4m3 DoubleRow matmul, fp32 PSUM):
  S[b, l] = <x_norm[b], memory[c, l]> * FP8_SCALE^2      [1024, 2048]
  per 512-col PSUM bank: E = exp(S / (FP8_SCALE^2 * T))  (ACT, bf16)
    - btiles 0,1: with free-dim accumulate -> zin partials
  cand = top-8 of each 256-wide chunk of E -> 64 values/sample (DVE MAX8)

Matmul schedule: btiles 0,1 accumulate into the two PSUM buffers with
matmuls interleaved in input-chunk-arrival order (the input DMA is the
gate early on); btiles 2-7 run bank-major (all 8 contraction chunks of
one 512-col bank back to back) so each PSUM bank is released to the
next btile as early as possible via its per-bank exp.

Host merge:
  epos[c, b] = exp(<x8[b], m8[c, tgt_b]>/T') recomputed in f32 from the
  exact fp8 operands the device consumed; intra CE = log(zin) - log(epos)
  on the own-camera core (host fallback matvec for the vanishingly rare
  cnt_c > 256 overflow rows).  For the inter loss the positive's value is
  removed from its camera's candidate list (nearest match to epos), the
  8x64 candidates are merged, and the exact top-50 negatives feed the
  log-sum-exp.  A global top-50 element can only be missing from the
  candidates if >=8 larger elements share its 256-chunk (P ~ 1e-5 per
  run, and the substitute is the next-ranked value, so the effect is
  ~1e-6 relative even then).
"""

import numpy as np

T = 0.05
HARD_NEG_K = 50
LOSS_WEIGHT = 0.5
N_CAMS = 8
L = 2048
D = 2048
B = 1024
NBT = 8          # batch tiles of 128
KC8 = 8          # contraction chunks of 256 (fp8 DoubleRow: 2 k-rows/cell)
FP8_SCALE = 32.0  # pre-scale before e4m3 cast (keeps values out of denormals)
NCH = 8          # candidate chunks per row
CHW = 256        # chunk width
NTOP = NCH * 8   # candidates shipped per camera (top-8 of each chunk)
NZT = 2          # btiles with exp+zin (own-camera rows live in tiles 0-1)
NWARM = 6        # PE warm-up matmuls (~2.6us; first input chunk lands ~11us)

_CACHE = {}


def _split_multi_waits(nc):
    """This container's walrus build rejects instructions carrying more than
    one sync wait ('Too many sync wait commands'). Hoist all but the last
    wait of each instruction onto same-engine Drain carriers placed just
    before it — semantically identical on an in-order engine stream."""
    import concourse.mybir as mybir

    n = 0
    for fn in nc.m.functions:
        for bb in fn.blocks:
            out = []
            for inst in bb.instructions:
                si = inst.sync_info
                if si is not None and si.on_wait and len(si.on_wait) > 1:
                    waits = list(si.on_wait)
                    for w in waits[:-1]:
                        d = mybir.InstDrain(name=f"ws-{n}", ins=[], outs=[])
                        n += 1
                        d.engine = inst.engine
                        d.sync_info = mybir.SyncInfo(on_wait=[w], on_update=[])
                        out.append(d)
                    si.on_wait = [waits[-1]]
                out.append(inst)
            if n:
                bb.instructions = out


def _build():
    import concourse.bass as bass
    import concourse.mybir as mybir
    from concourse import tile

    f32 = mybir.dt.float32
    bf16 = mybir.dt.bfloat16
    f8 = mybir.dt.float8e4
    Act = mybir.ActivationFunctionType

    nc = bass.Bass()
    # x and m packed per contraction chunk into one tensor: one DMA per
    # chunk (768KB) instead of two halves the serial-queue turnaround count
    xm = nc.dram_tensor("xm", [KC8, 128, 2, B + L], f8, kind="ExternalInput")
    zin_d = nc.dram_tensor("zin", [128, NZT], f32, kind="ExternalOutput")
    topv_d = nc.dram_tensor("topv", [NBT, 128, NTOP], bf16, kind="ExternalOutput")

    ESCALE = 1.0 / (FP8_SCALE * FP8_SCALE * T)

    with tile.TileContext(nc) as tc:
        with (
            tc.tile_pool(name="const", bufs=1) as cpool,
            tc.tile_pool(name="psum", bufs=8, space="PSUM") as ppool,
            tc.tile_pool(name="work", bufs=3) as wpool,
            tc.tile_pool(name="small", bufs=3) as spool,
        ):
            XM = cpool.tile([128, KC8, 2, B + L], f8)
            # All input transfers land on one HW queue and are serviced
            # serially in trigger order, so chunk order IS the arrival
            # order: chunk k lands just ahead of the fill matmuls that
            # consume it.
            for kc in range(KC8):
                nc.sync.dma_start(XM[:, kc, :, :], xm[kc])
            ZIN = cpool.tile([128, NZT], f32)

            # PE warm-up: HAM needs ~3-4us of sustained activity to reach
            # 2.4 GHz. Run throwaway matmuls on a zeroed scratch tile while
            # the input DMAs are still in flight; sized so the warm-up ends
            # about when the first input chunk pair lands (~13us).
            GB = cpool.tile([128, 640], f8)
            nc.vector.memset(GB[:], 0.0)
            WARM = ppool.tile([128, 512], f32, tag="S")
            for _ in range(NWARM):
                nc.tensor.matmul(
                    WARM[:], GB[:, 0:128], GB[:, 128:640],
                    start=True, stop=True,
                )

            # PSUM is managed as 8 single-bank [128,512] tiles: a btile's S
            # occupies 4 of them, and the pool rotation makes btile j+2's
            # bank-k matmuls wait only on the exp of btile j's bank k (not
            # on the whole previous tile's reader set).
            def s_banks():
                return [
                    ppool.tile([128, 512], f32, tag="S", name=f"sb{k}")
                    for k in range(4)
                ]

            # btiles 0 and 1 accumulate into both PSUM tile groups with
            # their matmuls interleaved in chunk-arrival order: during the
            # input DMA fill each arriving (X,M) chunk feeds 8 ready matmuls
            S_a = s_banks()
            S_b = s_banks()
            S_pair = [S_a, S_b]
            for i in range(KC8):
                for bt in range(2):
                    for nch in range(4):
                        nc.tensor.matmul(
                            S_pair[bt][nch][:],
                            XM[:, i, :, bt * 128 : (bt + 1) * 128],
                            XM[:, i, :, B + nch * 512 : B + (nch + 1) * 512],
                            start=(i == 0),
                            stop=(i == KC8 - 1),
                            perf_mode=mybir.MatmulPerfMode.DoubleRow,
                        )

            def exp_and_mine(S, bt):
                # per-512-bank exp (ACT, no accumulator -> no READ_ACC
                # serialization chain) so PSUM banks free progressively;
                # zin for the own-camera btiles 0-1 comes from a DVE
                # free-dim reduce over E instead.
                E = wpool.tile([128, L], bf16, tag="E")
                for nch in range(4):
                    sl = slice(nch * 512, (nch + 1) * 512)
                    nc.scalar.activation(
                        E[:, sl], S[nch][:], Act.Exp, scale=ESCALE,
                    )
                cand = spool.tile([128, NCH * 8], bf16, tag="cand")
                for ch in range(NCH):
                    nc.vector.max(
                        cand[:, ch * 8 : (ch + 1) * 8],
                        E[:, ch * CHW : (ch + 1) * CHW],
                    )
                nc.sync.dma_start(topv_d[bt], cand[:])
                if bt < NZT:
                    nc.vector.reduce_sum(
                        ZIN[:, bt : bt + 1], E[:], axis=mybir.AxisListType.X
                    )
                    if bt == NZT - 1:
                        nc.sync.dma_start(zin_d[:], ZIN[:])

            exp_and_mine(S_a, 0)
            exp_and_mine(S_b, 1)

            # btiles 2-7: bank-major matmul order (all 8 contraction chunks
            # of one 512-col bank back to back) so each bank's exp can fire
            # ~1.7us after the btile starts, releasing PSUM to btile+2
            # exactly when its matmuls reach that bank.
            for bt in range(2, NBT):
                S = s_banks()
                for nch in range(4):
                    for i, kc in enumerate([(k + bt) % KC8 for k in range(KC8)]):
                        nc.tensor.matmul(
                            S[nch][:],
                            XM[:, kc, :, bt * 128 : (bt + 1) * 128],
                            XM[:, kc, :, B + nch * 512 : B + (nch + 1) * 512],
                            start=(i == 0),
                            stop=(i == KC8 - 1),
                            perf_mode=mybir.MatmulPerfMode.DoubleRow,
                        )
                exp_and_mine(S, bt)

    _split_multi_waits(nc)
    return nc


def _get_nc():
    if "nc" not in _CACHE:
        _CACHE["nc"] = _build()
    return _CACHE["nc"]


def _pack_fp8(aT, ncols, f8):
    # [D, n] -> [KC8, 128, 2, n] with d = kc*256 + j*128 + p
    v = np.clip(aT * FP8_SCALE, -240.0, 240.0)
    v = v.reshape(KC8, 2, 128, ncols).transpose(0, 2, 1, 3)
    return np.ascontiguousarray(v).astype(f8)


def _prepare(inputs, memory, indexes, cams_all, labels_all):
    import ml_dtypes

    f8 = ml_dtypes.float8_e4m3
    inputs = np.asarray(inputs, np.float32)
    memory = np.asarray(memory, np.float32)
    indexes = np.asarray(indexes).astype(np.int64)
    cams_all = np.asarray(cams_all).astype(np.int64)
    cams = cams_all[indexes]

    x = inputs / np.linalg.norm(inputs, axis=1, keepdims=True)
    # per-core batch permutation: own-camera samples first (stable order)
    perms = [np.argsort(cams != c, kind="stable") for c in range(N_CAMS)]
    in_maps = []
    for c in range(N_CAMS):
        aT = np.concatenate([x[perms[c]].T, memory[c].T], axis=1)  # [D, B+L]
        in_maps.append({"xm": _pack_fp8(aT, B + L, f8)})
    return in_maps, perms, cams


def kernel(inputs, memory, indexes, cams_all, labels_all):
    from concourse.bass_utils import run_bass_kernel_spmd

    indexes = np.asarray(indexes).astype(np.int64)
    cams_all = np.asarray(cams_all).astype(np.int64)
    labels_all = np.asarray(labels_all).astype(np.int64)

    in_maps, perms, cams = _prepare(inputs, memory, indexes, cams_all, labels_all)
    nc = _get_nc()
    res = run_bass_kernel_spmd(nc, in_maps, list(range(N_CAMS)))

    # epos = exp(S[t]/T) computed host-side from the same fp8-quantized
    # inputs the device consumed (f32 arithmetic ~= PSUM fp32 accumulate).
    # x8/m8 reconstructed in the ORIGINAL batch order.
    tgts = labels_all[indexes]
    x_norm = np.asarray(inputs, np.float32)
    x_norm = x_norm / np.linalg.norm(x_norm, axis=1, keepdims=True)
    x8 = np.clip(x_norm.T * FP8_SCALE, -240.0, 240.0)
    x8 = x8.astype(in_maps[0]["xm"].dtype).astype(np.float32)   # [D, B]
    epos = np.empty((N_CAMS, B), np.float64)
    m8s = []
    for c in range(N_CAMS):
        m8 = (
            in_maps[c]["xm"].transpose(0, 2, 1, 3).reshape(D, B + L)[:, B:]
            .astype(np.float32)
        )
        m8s.append(m8)
        mt = m8[:, tgts]                     # [D, B]
        s_t = np.einsum("db,db->b", x8, mt, optimize=True)
        epos[c] = np.exp(s_t.astype(np.float64) / (FP8_SCALE * FP8_SCALE * T))

    bidx = np.arange(B)

    # gather per-core stats; zin rows are the first 2*128 rows of core c's
    # permuted batch; topv rows map back through the permutation
    zin_dev = np.empty((N_CAMS, NZT * 128), np.float64)
    topv = np.empty((N_CAMS, B, NTOP), np.float64)
    for c in range(N_CAMS):
        r = res.results[c]
        zin_dev[c] = r["zin"].astype(np.float64).T.reshape(NZT * 128)
        tv = r["topv"].astype(np.float64).reshape(B, NTOP)   # permuted rows
        inv = np.empty(B, np.int64)
        inv[perms[c]] = bidx
        topv[c] = tv[inv]                                    # original order

    # ---- intra: CE against own camera, mean within camera group, summed
    zin_own = np.empty(B, np.float64)
    for c in range(N_CAMS):
        own = np.flatnonzero(cams == c)                      # == perms[c][:cnt]
        rows = np.empty(B, np.int64)
        rows[perms[c]] = bidx                                # row of b in perm order
        r_own = rows[own]
        ok = r_own < NZT * 128
        zin_own[own[ok]] = zin_dev[c][r_own[ok]]
        for b in own[~ok]:                                   # overflow fallback
            s_row = x8[:, b] @ m8s[c]
            zin_own[b] = np.exp(
                s_row.astype(np.float64) / (FP8_SCALE * FP8_SCALE * T)
            ).sum()
    epos_own = epos[cams, bidx]
    ce = np.log(zin_own) - np.log(epos_own)
    cnt = np.bincount(cams, minlength=N_CAMS).astype(np.float64)
    ce_sum = np.bincount(cams, weights=ce, minlength=N_CAMS)
    loss_intra = np.sum(ce_sum / np.maximum(cnt, 1.0))

    # remove the positive's own value from each camera's candidate list:
    # nearest candidate within 0.5% of the host-computed epos (device values
    # are bf16-rounded, so exact equality is not available)
    for c in range(N_CAMS):
        relerr = np.abs(topv[c] - epos[c][:, None]) / epos[c][:, None]
        j = np.argmin(relerr, axis=1)
        hit = relerr[bidx, j] < 5e-3
        topv[c][bidx[hit], j[hit]] = 0.0

    # ---- inter: exact global top-50 negatives from 8x64 candidates
    cand = topv.transpose(1, 0, 2).reshape(B, N_CAMS * NTOP)
    part = np.partition(cand, cand.shape[1] - HARD_NEG_K, axis=1)
    z50 = part[:, cand.shape[1] - HARD_NEG_K :].sum(axis=1)
    sum_epos = epos.sum(axis=0)
    lse = np.log(sum_epos + z50)
    mean_logpos = np.log(epos).mean(axis=0)
    per_sample = lse - mean_logpos
    inter_sum = np.bincount(cams, weights=per_sample, minlength=N_CAMS)
    loss_inter = np.sum(inter_sum / np.maximum(cnt, 1.0)) * LOSS_WEIGHT

    return np.float32(loss_intra), np.float32(loss_inter)


# revision 25
# speedup vs baseline: 1.0919x; 1.0116x over previous
"""CAPMemory loss kernel for 8 Trainium2 NeuronCores.

Sharding: camera-sharded -- core c owns memory[c], the batch is replicated
(the per-sample stats each core produces are tiny, so this moves 16x less
HBM traffic than batch-sharding the replicated 128 MiB memory bank).

Per-core batch permutation: core c receives the batch reordered so its
own-camera samples occupy rows 0..cnt_c-1 (cnt_c <= 256 w.h.p.).  The
intra-loss denominator zin = sum_l exp(S/T) is then only needed for batch
tiles 0-1, so the device computes exp for those two tiles only.

Device, per core (fp8 e4m3 DoubleRow matmul, fp32 PSUM):
  S[b, l] = <x_norm[b], memory[c, l]> * FP8_SCALE^2      [1024, 2048]
  per 512-col PSUM bank: E = exp(S / (FP8_SCALE^2 * T))  (ACT, bf16)
    - btiles 0,1: with free-dim accumulate -> zin partials
  cand = top-8 of each 256-wide chunk of E -> 64 values/sample (DVE MAX8)

Matmul schedule: btiles 0,1 accumulate into the two PSUM buffers with
matmuls interleaved in input-chunk-arrival order (the input DMA is the
gate early on); btiles 2-7 run bank-major (all 8 contraction chunks of
one 512-col bank back to back) so each PSUM bank is released to the
next btile as early as possible via its per-bank exp.

Host merge:
  epos[c, b] = exp(<x8[b], m8[c, tgt_b]>/T') recomputed in f32 from the
  exact fp8 operands the device consumed; intra CE = log(zin) - log(epos)
  on the own-camera core (host fallback matvec for the vanishingly rare
  cnt_c > 256 overflow rows).  For the inter loss the positive's value is
  removed from its camera's candidate list (nearest match to epos), the
  8x64 candidates are merged, and the exact top-50 negatives feed the
  log-sum-exp.  A global top-50 element can only be missing from the
  candidates if >=8 larger elements share its 256-chunk (P ~ 1e-5 per
  run, and the substitute is the next-ranked value, so the effect is
  ~1e-6 relative even then).
"""

import numpy as np

T = 0.05
HARD_NEG_K = 50
LOSS_WEIGHT = 0.5
N_CAMS = 8
L = 2048
D = 2048
B = 1024
NBT = 8          # batch tiles of 128
KC8 = 8          # contraction chunks of 256 (fp8 DoubleRow: 2 k-rows/cell)
FP8_SCALE = 32.0  # pre-scale before e4m3 cast (keeps values out of denormals)
NCH = 8          # candidate chunks per row
CHW = 256        # chunk width
NTOP = NCH * 8   # candidates shipped per camera (top-8 of each chunk)
NZT = 2          # btiles with exp+zin (own-camera rows live in tiles 0-1)
NWARM = 8        # PE warm-up matmuls (~3.4us; first input chunks land ~13us)

_CACHE = {}


def _split_multi_waits(nc):
    """This container's walrus build rejects instructions carrying more than
    one sync wait ('Too many sync wait commands'). Hoist all but the last
    wait of each instruction onto same-engine Drain carriers placed just
    before it — semantically identical on an in-order engine stream."""
    import concourse.mybir as mybir

    n = 0
    for fn in nc.m.functions:
        for bb in fn.blocks:
            out = []
            for inst in bb.instructions:
                si = inst.sync_info
                if si is not None and si.on_wait and len(si.on_wait) > 1:
                    waits = list(si.on_wait)
                    for w in waits[:-1]:
                        d = mybir.InstDrain(name=f"ws-{n}", ins=[], outs=[])
                        n += 1
                        d.engine = inst.engine
                        d.sync_info = mybir.SyncInfo(on_wait=[w], on_update=[])
                        out.append(d)
                    si.on_wait = [waits[-1]]
                out.append(inst)
            if n:
                bb.instructions = out


def _build():
    import concourse.bass as bass
    import concourse.mybir as mybir
    from concourse import tile

    f32 = mybir.dt.float32
    bf16 = mybir.dt.bfloat16
    f8 = mybir.dt.float8e4
    Act = mybir.ActivationFunctionType

    nc = bass.Bass()
    # x and m packed per contraction chunk into one tensor: one DMA per
    # chunk (768KB) instead of two halves the serial-queue turnaround count
    xm = nc.dram_tensor("xm", [KC8, 128, 2, B + L], f8, kind="ExternalInput")
    zin_d = nc.dram_tensor("zin", [128, NZT], f32, kind="ExternalOutput")
    topv_d = nc.dram_tensor("topv", [NBT, 128, NTOP], bf16, kind="ExternalOutput")

    ESCALE = 1.0 / (FP8_SCALE * FP8_SCALE * T)

    with tile.TileContext(nc) as tc:
        with (
            tc.tile_pool(name="const", bufs=1) as cpool,
            tc.tile_pool(name="psum", bufs=8, space="PSUM") as ppool,
            tc.tile_pool(name="work", bufs=3) as wpool,
            tc.tile_pool(name="small", bufs=3) as spool,
        ):
            XM = cpool.tile([128, KC8, 2, B + L], f8)
            # All input transfers land on one HW queue and are serviced
            # serially in trigger order, so issue order IS the arrival
            # order.  Two chunks per transfer: few enough transfers that
            # the per-transfer completion-receipt bubbles stay small, small
            # enough that arrivals stay just ahead of the fill matmuls.
            for g in range(KC8 // 2):
                nc.sync.dma_start(
                    XM[:, 2 * g : 2 * g + 2, :, :], xm[2 * g : 2 * g + 2]
                )
            ZIN = cpool.tile([128, NZT], f32)

            # PE warm-up: HAM needs ~3-4us of sustained activity to reach
            # 2.4 GHz. Run throwaway matmuls on a zeroed scratch tile while
            # the input DMAs are still in flight; sized so the warm-up ends
            # about when the first input chunk pair lands (~13us).
            GB = cpool.tile([128, 640], f8)
            nc.vector.memset(GB[:], 0.0)
            WARM = ppool.tile([128, 512], f32, tag="S")
            for _ in range(NWARM):
                nc.tensor.matmul(
                    WARM[:], GB[:, 0:128], GB[:, 128:640],
                    start=True, stop=True,
                )

            # PSUM is managed as 8 single-bank [128,512] tiles: a btile's S
            # occupies 4 of them, and the pool rotation makes btile j+2's
            # bank-k matmuls wait only on the exp of btile j's bank k (not
            # on the whole previous tile's reader set).
            def s_banks():
                return [
                    ppool.tile([128, 512], f32, tag="S", name=f"sb{k}")
                    for k in range(4)
                ]

            # btiles 0 and 1 accumulate into both PSUM tile groups with
            # their matmuls interleaved in chunk-arrival order: during the
            # input DMA fill each arriving (X,M) chunk feeds 8 ready matmuls
            S_a = s_banks()
            S_b = s_banks()
            S_pair = [S_a, S_b]
            for i in range(KC8):
                for bt in range(2):
                    for nch in range(4):
                        nc.tensor.matmul(
                            S_pair[bt][nch][:],
                            XM[:, i, :, bt * 128 : (bt + 1) * 128],
                            XM[:, i, :, B + nch * 512 : B + (nch + 1) * 512],
                            start=(i == 0),
                            stop=(i == KC8 - 1),
                            perf_mode=mybir.MatmulPerfMode.DoubleRow,
                        )

            def exp_and_mine(S, bt):
                # per-512-bank exp (ACT, no accumulator -> no READ_ACC
                # serialization chain) so PSUM banks free progressively;
                # zin for the own-camera btiles 0-1 comes from a DVE
                # free-dim reduce over E instead.
                E = wpool.tile([128, L], bf16, tag="E")
                for nch in range(4):
                    sl = slice(nch * 512, (nch + 1) * 512)
                    nc.scalar.activation(
                        E[:, sl], S[nch][:], Act.Exp, scale=ESCALE,
                    )
                cand = spool.tile([128, NCH * 8], bf16, tag="cand")
                for ch in range(NCH):
                    nc.vector.max(
                        cand[:, ch * 8 : (ch + 1) * 8],
                        E[:, ch * CHW : (ch + 1) * CHW],
                    )
                nc.sync.dma_start(topv_d[bt], cand[:])
                if bt < NZT:
                    nc.vector.reduce_sum(
                        ZIN[:, bt : bt + 1], E[:], axis=mybir.AxisListType.X
                    )
                    if bt == NZT - 1:
                        nc.sync.dma_start(zin_d[:], ZIN[:])

            exp_and_mine(S_a, 0)
            exp_and_mine(S_b, 1)

            # btiles 2-7: bank-major matmul order (all 8 contraction chunks
            # of one 512-col bank back to back) so each bank's exp can fire
            # ~1.7us after the btile starts, releasing PSUM to btile+2
            # exactly when its matmuls reach that bank.
            for bt in range(2, NBT):
                S = s_banks()
                for nch in range(4):
                    for i, kc in enumerate([(k + bt) % KC8 for k in range(KC8)]):
                        nc.tensor.matmul(
                            S[nch][:],
                            XM[:, kc, :, bt * 128 : (bt + 1) * 128],
                            XM[:, kc, :, B + nch * 512 : B + (nch + 1) * 512],
                            start=(i == 0),
                            stop=(i == KC8 - 1),
                            perf_mode=mybir.MatmulPerfMode.DoubleRow,
                        )
                exp_and_mine(S, bt)

    _split_multi_waits(nc)
    return nc


def _get_nc():
    if "nc" not in _CACHE:
        _CACHE["nc"] = _build()
    return _CACHE["nc"]


def _pack_fp8(aT, ncols, f8):
    # [D, n] -> [KC8, 128, 2, n] with d = kc*256 + j*128 + p
    v = np.clip(aT * FP8_SCALE, -240.0, 240.0)
    v = v.reshape(KC8, 2, 128, ncols).transpose(0, 2, 1, 3)
    return np.ascontiguousarray(v).astype(f8)


def _prepare(inputs, memory, indexes, cams_all, labels_all):
    import ml_dtypes

    f8 = ml_dtypes.float8_e4m3
    inputs = np.asarray(inputs, np.float32)
    memory = np.asarray(memory, np.float32)
    indexes = np.asarray(indexes).astype(np.int64)
    cams_all = np.asarray(cams_all).astype(np.int64)
    cams = cams_all[indexes]

    x = inputs / np.linalg.norm(inputs, axis=1, keepdims=True)
    # per-core batch permutation: own-camera samples first (stable order)
    perms = [np.argsort(cams != c, kind="stable") for c in range(N_CAMS)]
    in_maps = []
    for c in range(N_CAMS):
        aT = np.concatenate([x[perms[c]].T, memory[c].T], axis=1)  # [D, B+L]
        in_maps.append({"xm": _pack_fp8(aT, B + L, f8)})
    return in_maps, perms, cams


def kernel(inputs, memory, indexes, cams_all, labels_all):
    from concourse.bass_utils import run_bass_kernel_spmd

    indexes = np.asarray(indexes).astype(np.int64)
    cams_all = np.asarray(cams_all).astype(np.int64)
    labels_all = np.asarray(labels_all).astype(np.int64)

    in_maps, perms, cams = _prepare(inputs, memory, indexes, cams_all, labels_all)
    nc = _get_nc()
    res = run_bass_kernel_spmd(nc, in_maps, list(range(N_CAMS)))

    # epos = exp(S[t]/T) computed host-side from the same fp8-quantized
    # inputs the device consumed (f32 arithmetic ~= PSUM fp32 accumulate).
    # x8/m8 reconstructed in the ORIGINAL batch order.
    tgts = labels_all[indexes]
    x_norm = np.asarray(inputs, np.float32)
    x_norm = x_norm / np.linalg.norm(x_norm, axis=1, keepdims=True)
    x8 = np.clip(x_norm.T * FP8_SCALE, -240.0, 240.0)
    x8 = x8.astype(in_maps[0]["xm"].dtype).astype(np.float32)   # [D, B]
    epos = np.empty((N_CAMS, B), np.float64)
    m8s = []
    for c in range(N_CAMS):
        m8 = (
            in_maps[c]["xm"].transpose(0, 2, 1, 3).reshape(D, B + L)[:, B:]
            .astype(np.float32)
        )
        m8s.append(m8)
        mt = m8[:, tgts]                     # [D, B]
        s_t = np.einsum("db,db->b", x8, mt, optimize=True)
        epos[c] = np.exp(s_t.astype(np.float64) / (FP8_SCALE * FP8_SCALE * T))

    bidx = np.arange(B)

    # gather per-core stats; zin rows are the first 2*128 rows of core c's
    # permuted batch; topv rows map back through the permutation
    zin_dev = np.empty((N_CAMS, NZT * 128), np.float64)
    topv = np.empty((N_CAMS, B, NTOP), np.float64)
    for c in range(N_CAMS):
        r = res.results[c]
        zin_dev[c] = r["zin"].astype(np.float64).T.reshape(NZT * 128)
        tv = r["topv"].astype(np.float64).reshape(B, NTOP)   # permuted rows
        inv = np.empty(B, np.int64)
        inv[perms[c]] = bidx
        topv[c] = tv[inv]                                    # original order

    # ---- intra: CE against own camera, mean within camera group, summed
    zin_own = np.empty(B, np.float64)
    for c in range(N_CAMS):
        own = np.flatnonzero(cams == c)                      # == perms[c][:cnt]
        rows = np.empty(B, np.int64)
        rows[perms[c]] = bidx                                # row of b in perm order
        r_own = rows[own]
        ok = r_own < NZT * 128
        zin_own[own[ok]] = zin_dev[c][r_own[ok]]
        for b in own[~ok]:                                   # overflow fallback
            s_row = x8[:, b] @ m8s[c]
            zin_own[b] = np.exp(
                s_row.astype(np.float64) / (FP8_SCALE * FP8_SCALE * T)
            ).sum()
    epos_own = epos[cams, bidx]
    ce = np.log(zin_own) - np.log(epos_own)
    cnt = np.bincount(cams, minlength=N_CAMS).astype(np.float64)
    ce_sum = np.bincount(cams, weights=ce, minlength=N_CAMS)
    loss_intra = np.sum(ce_sum / np.maximum(cnt, 1.0))

    # remove the positive's own value from each camera's candidate list:
    # nearest candidate within 0.5% of the host-computed epos (device values
    # are bf16-rounded, so exact equality is not available)
    for c in range(N_CAMS):
        relerr = np.abs(topv[c] - epos[c][:, None]) / epos[c][:, None]
        j = np.argmin(relerr, axis=1)
        hit = relerr[bidx, j] < 5e-3
        topv[c][bidx[hit], j[hit]] = 0.0

    # ---- inter: exact global top-50 negatives from 8x64 candidates
    cand = topv.transpose(1, 0, 2).reshape(B, N_CAMS * NTOP)
    part = np.partition(cand, cand.shape[1] - HARD_NEG_K, axis=1)
    z50 = part[:, cand.shape[1] - HARD_NEG_K :].sum(axis=1)
    sum_epos = epos.sum(axis=0)
    lse = np.log(sum_epos + z50)
    mean_logpos = np.log(epos).mean(axis=0)
    per_sample = lse - mean_logpos
    inter_sum = np.bincount(cams, weights=per_sample, minlength=N_CAMS)
    loss_inter = np.sum(inter_sum / np.maximum(cnt, 1.0)) * LOSS_WEIGHT

    return np.float32(loss_intra), np.float32(loss_inter)


# revision 28
# speedup vs baseline: 1.1106x; 1.0171x over previous
"""CAPMemory loss kernel for 8 Trainium2 NeuronCores.

Sharding: camera-sharded -- core c owns memory[c], the batch is replicated
(the per-sample stats each core produces are tiny, so this moves 16x less
HBM traffic than batch-sharding the replicated 128 MiB memory bank).

Per-core batch permutation: core c receives the batch reordered so its
own-camera samples occupy rows 0..cnt_c-1 (cnt_c <= 256 w.h.p.).  The
intra-loss denominator zin = sum_l exp(S/T) is then only needed for batch
tiles 0-1, so the device computes exp for those two tiles only.

Device, per core (fp8 e4m3 DoubleRow matmul, fp32 PSUM):
  S[b, l] = <x_norm[b], memory[c, l]> * FP8_SCALE^2      [1024, 2048]
  per 512-col PSUM bank: E = exp(S / (FP8_SCALE^2 * T))  (ACT, bf16)
    - btiles 0,1: with free-dim accumulate -> zin partials
  cand = top-8 of each 256-wide chunk of E -> 64 values/sample (DVE MAX8)

Matmul schedule: btiles 0,1 accumulate into the two PSUM buffers with
matmuls interleaved in input-chunk-arrival order (the input DMA is the
gate early on); btiles 2-7 run bank-major (all 8 contraction chunks of
one 512-col bank back to back) so each PSUM bank is released to the
next btile as early as possible via its per-bank exp.

Host merge:
  epos[c, b] = exp(<x8[b], m8[c, tgt_b]>/T') recomputed in f32 from the
  exact fp8 operands the device consumed; intra CE = log(zin) - log(epos)
  on the own-camera core (host fallback matvec for the vanishingly rare
  cnt_c > 256 overflow rows).  For the inter loss the positive's value is
  removed from its camera's candidate list (nearest match to epos), the
  8x64 candidates are merged, and the exact top-50 negatives feed the
  log-sum-exp.  A global top-50 element can only be missing from the
  candidates if >=8 larger elements share its 256-chunk (P ~ 1e-5 per
  run, and the substitute is the next-ranked value, so the effect is
  ~1e-6 relative even then).
"""

import numpy as np

T = 0.05
HARD_NEG_K = 50
LOSS_WEIGHT = 0.5
N_CAMS = 8
L = 2048
D = 2048
B = 1024
NBT = 8          # batch tiles of 128
KC8 = 8          # contraction chunks of 256 (fp8 DoubleRow: 2 k-rows/cell)
FP8_SCALE = 32.0  # pre-scale before e4m3 cast (keeps values out of denormals)
NCH = 8          # candidate chunks per row
CHW = 256        # chunk width
NTOP = NCH * 8   # candidates shipped per camera (top-8 of each chunk)
NZT = 2          # btiles with exp+zin (own-camera rows live in tiles 0-1)
NWARM = 12       # PE warm-up matmuls (~4.3us; first input chunks land ~13us)

_CACHE = {}


def _split_multi_waits(nc):
    """This container's walrus build rejects instructions carrying more than
    one sync wait ('Too many sync wait commands'). Hoist all but the last
    wait of each instruction onto same-engine Drain carriers placed just
    before it — semantically identical on an in-order engine stream."""
    import concourse.mybir as mybir

    n = 0
    for fn in nc.m.functions:
        for bb in fn.blocks:
            out = []
            for inst in bb.instructions:
                si = inst.sync_info
                if si is not None and si.on_wait and len(si.on_wait) > 1:
                    waits = list(si.on_wait)
                    for w in waits[:-1]:
                        d = mybir.InstDrain(name=f"ws-{n}", ins=[], outs=[])
                        n += 1
                        d.engine = inst.engine
                        d.sync_info = mybir.SyncInfo(on_wait=[w], on_update=[])
                        out.append(d)
                    si.on_wait = [waits[-1]]
                out.append(inst)
            if n:
                bb.instructions = out


def _build():
    import concourse.bass as bass
    import concourse.mybir as mybir
    from concourse import tile

    f32 = mybir.dt.float32
    bf16 = mybir.dt.bfloat16
    f8 = mybir.dt.float8e4
    Act = mybir.ActivationFunctionType

    nc = bass.Bass()
    # x and m packed per contraction chunk into one tensor: one DMA per
    # chunk (768KB) instead of two halves the serial-queue turnaround count
    xm = nc.dram_tensor("xm", [KC8, 128, 2, B + L], f8, kind="ExternalInput")
    zin_d = nc.dram_tensor("zin", [128, NZT], f32, kind="ExternalOutput")
    topv_d = nc.dram_tensor("topv", [NBT, 128, NTOP], bf16, kind="ExternalOutput")

    ESCALE = 1.0 / (FP8_SCALE * FP8_SCALE * T)

    with tile.TileContext(nc) as tc:
        with (
            tc.tile_pool(name="const", bufs=1) as cpool,
            tc.tile_pool(name="psum", bufs=8, space="PSUM") as ppool,
            tc.tile_pool(name="work", bufs=3) as wpool,
            tc.tile_pool(name="small", bufs=3) as spool,
        ):
            XM = cpool.tile([128, KC8, 2, B + L], f8)
            # All input transfers land on one HW queue and are serviced
            # serially in trigger order, so issue order IS the arrival
            # order.  Two chunks per transfer: few enough transfers that
            # the per-transfer completion-receipt bubbles stay small, small
            # enough that arrivals stay just ahead of the fill matmuls.
            for lo, hi in ((0, 2), (2, 4), (4, 6), (6, 7), (7, 8)):
                nc.sync.dma_start(XM[:, lo:hi, :, :], xm[lo:hi])
            ZIN = cpool.tile([128, NZT], f32)

            # PE warm-up: HAM needs ~3-4us of sustained activity to reach
            # 2.4 GHz. Run throwaway matmuls on a zeroed scratch tile while
            # the input DMAs are still in flight; sized so the warm-up ends
            # about when the first input chunk pair lands (~13us).
            GB = cpool.tile([128, 640], f8)
            nc.vector.memset(GB[:], 0.0)
            WARM = ppool.tile([128, 512], f32, tag="S")
            for _ in range(NWARM):
                nc.tensor.matmul(
                    WARM[:], GB[:, 0:128], GB[:, 128:640],
                    start=True, stop=True,
                )

            # PSUM is managed as 8 single-bank [128,512] tiles: a btile's S
            # occupies 4 of them, and the pool rotation makes btile j+2's
            # bank-k matmuls wait only on the exp of btile j's bank k (not
            # on the whole previous tile's reader set).
            def s_banks():
                return [
                    ppool.tile([128, 512], f32, tag="S", name=f"sb{k}")
                    for k in range(4)
                ]

            # btiles 0 and 1 accumulate into both PSUM tile groups with
            # their matmuls interleaved in chunk-arrival order: during the
            # input DMA fill each arriving (X,M) chunk feeds 8 ready matmuls
            S_a = s_banks()
            S_b = s_banks()
            S_pair = [S_a, S_b]
            for i in range(KC8):
                for bt in range(2):
                    for nch in range(4):
                        nc.tensor.matmul(
                            S_pair[bt][nch][:],
                            XM[:, i, :, bt * 128 : (bt + 1) * 128],
                            XM[:, i, :, B + nch * 512 : B + (nch + 1) * 512],
                            start=(i == 0),
                            stop=(i == KC8 - 1),
                            perf_mode=mybir.MatmulPerfMode.DoubleRow,
                        )

            def exp_and_mine(S, bt):
                # per-512-bank exp (ACT, no accumulator -> no READ_ACC
                # serialization chain) so PSUM banks free progressively;
                # zin for the own-camera btiles 0-1 comes from a DVE
                # free-dim reduce over E instead.
                E = wpool.tile([128, L], bf16, tag="E")
                for nch in range(4):
                    sl = slice(nch * 512, (nch + 1) * 512)
                    if bt == NBT - 1 and nch == 3:
                        # last bank of the last btile: halve the exp so the
                        # first MAX8 starts ~0.3us sooner on the tail path
                        for hf in range(2):
                            hs = slice(nch * 512 + hf * 256, nch * 512 + (hf + 1) * 256)
                            nc.scalar.activation(
                                E[:, hs], S[nch][:, hf * 256 : (hf + 1) * 256],
                                Act.Exp, scale=ESCALE,
                            )
                    else:
                        nc.scalar.activation(
                            E[:, sl], S[nch][:], Act.Exp, scale=ESCALE,
                        )
                cand = spool.tile([128, NCH * 8], bf16, tag="cand")
                for ch in range(NCH):
                    nc.vector.max(
                        cand[:, ch * 8 : (ch + 1) * 8],
                        E[:, ch * CHW : (ch + 1) * CHW],
                    )
                nc.sync.dma_start(topv_d[bt], cand[:])
                if bt < NZT:
                    nc.vector.reduce_sum(
                        ZIN[:, bt : bt + 1], E[:], axis=mybir.AxisListType.X
                    )
                    if bt == NZT - 1:
                        nc.sync.dma_start(zin_d[:], ZIN[:])

            exp_and_mine(S_a, 0)
            exp_and_mine(S_b, 1)

            # btiles 2-7: bank-major matmul order (all 8 contraction chunks
            # of one 512-col bank back to back) so each bank's exp can fire
            # ~1.7us after the btile starts, releasing PSUM to btile+2
            # exactly when its matmuls reach that bank.
            for bt in range(2, NBT):
                S = s_banks()
                for nch in range(4):
                    for i, kc in enumerate([(k + bt) % KC8 for k in range(KC8)]):
                        nc.tensor.matmul(
                            S[nch][:],
                            XM[:, kc, :, bt * 128 : (bt + 1) * 128],
                            XM[:, kc, :, B + nch * 512 : B + (nch + 1) * 512],
                            start=(i == 0),
                            stop=(i == KC8 - 1),
                            perf_mode=mybir.MatmulPerfMode.DoubleRow,
                        )
                exp_and_mine(S, bt)

    _split_multi_waits(nc)
    return nc


def _get_nc():
    if "nc" not in _CACHE:
        _CACHE["nc"] = _build()
    return _CACHE["nc"]


def _pack_fp8(aT, ncols, f8):
    # [D, n] -> [KC8, 128, 2, n] with d = kc*256 + j*128 + p
    v = np.clip(aT * FP8_SCALE, -240.0, 240.0)
    v = v.reshape(KC8, 2, 128, ncols).transpose(0, 2, 1, 3)
    return np.ascontiguousarray(v).astype(f8)


def _prepare(inputs, memory, indexes, cams_all, labels_all):
    import ml_dtypes

    f8 = ml_dtypes.float8_e4m3
    inputs = np.asarray(inputs, np.float32)
    memory = np.asarray(memory, np.float32)
    indexes = np.asarray(indexes).astype(np.int64)
    cams_all = np.asarray(cams_all).astype(np.int64)
    cams = cams_all[indexes]

    x = inputs / np.linalg.norm(inputs, axis=1, keepdims=True)
    # per-core batch permutation: own-camera samples first (stable order)
    perms = [np.argsort(cams != c, kind="stable") for c in range(N_CAMS)]
    in_maps = []
    for c in range(N_CAMS):
        aT = np.concatenate([x[perms[c]].T, memory[c].T], axis=1)  # [D, B+L]
        in_maps.append({"xm": _pack_fp8(aT, B + L, f8)})
    return in_maps, perms, cams


def kernel(inputs, memory, indexes, cams_all, labels_all):
    from concourse.bass_utils import run_bass_kernel_spmd

    indexes = np.asarray(indexes).astype(np.int64)
    cams_all = np.asarray(cams_all).astype(np.int64)
    labels_all = np.asarray(labels_all).astype(np.int64)

    in_maps, perms, cams = _prepare(inputs, memory, indexes, cams_all, labels_all)
    nc = _get_nc()
    res = run_bass_kernel_spmd(nc, in_maps, list(range(N_CAMS)))

    # epos = exp(S[t]/T) computed host-side from the same fp8-quantized
    # inputs the device consumed (f32 arithmetic ~= PSUM fp32 accumulate).
    # x8/m8 reconstructed in the ORIGINAL batch order.
    tgts = labels_all[indexes]
    x_norm = np.asarray(inputs, np.float32)
    x_norm = x_norm / np.linalg.norm(x_norm, axis=1, keepdims=True)
    x8 = np.clip(x_norm.T * FP8_SCALE, -240.0, 240.0)
    x8 = x8.astype(in_maps[0]["xm"].dtype).astype(np.float32)   # [D, B]
    epos = np.empty((N_CAMS, B), np.float64)
    m8s = []
    for c in range(N_CAMS):
        m8 = (
            in_maps[c]["xm"].transpose(0, 2, 1, 3).reshape(D, B + L)[:, B:]
            .astype(np.float32)
        )
        m8s.append(m8)
        mt = m8[:, tgts]                     # [D, B]
        s_t = np.einsum("db,db->b", x8, mt, optimize=True)
        epos[c] = np.exp(s_t.astype(np.float64) / (FP8_SCALE * FP8_SCALE * T))

    bidx = np.arange(B)

    # gather per-core stats; zin rows are the first 2*128 rows of core c's
    # permuted batch; topv rows map back through the permutation
    zin_dev = np.empty((N_CAMS, NZT * 128), np.float64)
    topv = np.empty((N_CAMS, B, NTOP), np.float64)
    for c in range(N_CAMS):
        r = res.results[c]
        zin_dev[c] = r["zin"].astype(np.float64).T.reshape(NZT * 128)
        tv = r["topv"].astype(np.float64).reshape(B, NTOP)   # permuted rows
        inv = np.empty(B, np.int64)
        inv[perms[c]] = bidx
        topv[c] = tv[inv]                                    # original order

    # ---- intra: CE against own camera, mean within camera group, summed
    zin_own = np.empty(B, np.float64)
    for c in range(N_CAMS):
        own = np.flatnonzero(cams == c)                      # == perms[c][:cnt]
        rows = np.empty(B, np.int64)
        rows[perms[c]] = bidx                                # row of b in perm order
        r_own = rows[own]
        ok = r_own < NZT * 128
        zin_own[own[ok]] = zin_dev[c][r_own[ok]]
        for b in own[~ok]:                                   # overflow fallback
            s_row = x8[:, b] @ m8s[c]
            zin_own[b] = np.exp(
                s_row.astype(np.float64) / (FP8_SCALE * FP8_SCALE * T)
            ).sum()
    epos_own = epos[cams, bidx]
    ce = np.log(zin_own) - np.log(epos_own)
    cnt = np.bincount(cams, minlength=N_CAMS).astype(np.float64)
    ce_sum = np.bincount(cams, weights=ce, minlength=N_CAMS)
    loss_intra = np.sum(ce_sum / np.maximum(cnt, 1.0))

    # remove the positive's own value from each camera's candidate list:
    # nearest candidate within 0.5% of the host-computed epos (device values
    # are bf16-rounded, so exact equality is not available)
    for c in range(N_CAMS):
        relerr = np.abs(topv[c] - epos[c][:, None]) / epos[c][:, None]
        j = np.argmin(relerr, axis=1)
        hit = relerr[bidx, j] < 5e-3
        topv[c][bidx[hit], j[hit]] = 0.0

    # ---- inter: exact global top-50 negatives from 8x64 candidates
    cand = topv.transpose(1, 0, 2).reshape(B, N_CAMS * NTOP)
    part = np.partition(cand, cand.shape[1] - HARD_NEG_K, axis=1)
    z50 = part[:, cand.shape[1] - HARD_NEG_K :].sum(axis=1)
    sum_epos = epos.sum(axis=0)
    lse = np.log(sum_epos + z50)
    mean_logpos = np.log(epos).mean(axis=0)
    per_sample = lse - mean_logpos
    inter_sum = np.bincount(cams, weights=per_sample, minlength=N_CAMS)
    loss_inter = np.sum(inter_sum / np.maximum(cnt, 1.0)) * LOSS_WEIGHT

    return np.float32(loss_intra), np.float32(loss_inter)
